# revision 2
# baseline (speedup 1.0000x reference)
"""Trainium2 Bass kernel for nn_Encoder_Decoder: embedding + LSTM over
SEQ=256 steps, BATCH=128, HIDDEN=1024, returning all hidden states.

Strategy (data-parallel, 8 cores, batch 16 per core, no collectives):
  Phase 1 (parallel over time): gather embeddings for all (t, b), transpose
    on the PE to build X^T, then one big matmul A^T = W_ih @ X^T + bias in
    float32r (full-rate, ~tf32 precision), staged to DRAM scratch.
  Phase 2 (sequential over t): state kept transposed+packed as
    [128 partitions, 16q+b] so the LSTM elementwise needs no per-step
    transposes. Recurrent matmul W_hh @ h^T runs weight-stationary in bf16
    (256 small matmuls of N=16 per step). Gates = psum + A^T[t]; fused
    sigmoid/tanh on the scalar engine; c stays fp32.

SBUF is managed with a hand-drawn map (alloc_sbuf_tensor_at) — phase 2's
weight slab aliases phase 1's X^T slab across a strict barrier.
Output is written transposed/packed; the host reassembles to [256, 128, 1024].
"""

import sys

for _p in ("/opt/trn_rl_repo/concourse", "/opt/trn_rl_repo"):
    if _p not in sys.path:
        sys.path.insert(0, _p)

import os

import numpy as np
import ml_dtypes

REPEAT = int(os.environ.get("REPEAT", "1"))  # timing-only: repeat phase 2

SEQ, BATCH, HIDDEN, VOCAB = 256, 128, 1024, 50000
NCORES = 8
CB = BATCH // NCORES          # batch per core = 16
NH = HIDDEN // 128            # hidden k-tiles = 8
NJ = 4 * HIDDEN // 128        # gate j-tiles = 32

_cache = {}


def _build(seq):
    """Build (and cache) the Bass program for a given sequence length."""
    if seq in _cache:
        return _cache[seq]

    import concourse.bass as bass
    import concourse.mybir as mybir
    import concourse.tile as tile
    from concourse import bacc

    f32 = mybir.dt.float32
    f32r = mybir.dt.float32r
    bf16 = mybir.dt.bfloat16
    f8e4 = mybir.dt.float8e4
    i32 = mybir.dt.int32
    ACT = mybir.ActivationFunctionType
    WSCALE = 64.0  # W_hh pre-scaled by 64 into fp8 normal range; gates carried x64

    rows = seq * CB               # gathered rows per core
    nrt = rows // 128             # row tiles (32 at seq=256)
    CHUNK = min(512, rows)        # matmul moving-dim chunk
    ncc = rows // CHUNK           # column chunks (8 at seq=256)
    H4 = 4 * HIDDEN

    nc = bacc.Bacc("TRN2", target_bir_lowering=False, debug=False, num_devices=NCORES)

    idx_d = nc.dram_tensor("idx", [128, nrt], i32, kind="ExternalInput")
    embed_d = nc.dram_tensor("embed", [VOCAB, HIDDEN], f32r, kind="ExternalInput")
    wih_d = nc.dram_tensor("wihT", [HIDDEN, H4], f32r, kind="ExternalInput")
    whh_d = nc.dram_tensor("whhT", [HIDDEN, H4], f8e4, kind="ExternalInput")
    bias_d = nc.dram_tensor("biasT", [128, NJ], f32, kind="ExternalInput")
    ident_d = nc.dram_tensor("ident", [128, 128], f32r, kind="ExternalInput")
    out_d = nc.dram_tensor("out", [seq, 128, NH * CB], f32, kind="ExternalOutput")
    a_d = nc.dram_tensor("a_scratch", [seq, 128, NJ * CB], f32, kind="Internal")

    # ---------------- hand-drawn SBUF map (bytes per partition) -------------
    big_bytes = max(rows * NH * 4, H4 * NH * 2)   # xt (f32r) vs whh (bf16) slab
    wih_off = big_bytes                            # 2 slots x [128,1024] f32r
    xr_off = wih_off + 2 * HIDDEN * 4              # 2 slots x [128,1024] f32r
    blk_off = xr_off + 2 * HIDDEN * 4              # at slots / phase-2 block
    at_bytes = rows * 4
    p2_bytes = (4 * 512 + 2 * 512 + 2 * 512 + 512 + 2 * 1024 + 128) * 4 + 256 * 2 + 2048
    blk_bytes = max(2 * at_bytes, p2_bytes)
    const_off = blk_off + blk_bytes                # idx/bias/ident
    const_bytes = nrt * 4 + NJ * 4 + 128 * 4 + 256
    total = const_off + const_bytes

    arena = nc.alloc_sbuf_tensor("arena", [128, total], mybir.dt.uint8)
    base = nc.lookup_mloc(arena).addr

    def at_(name, shape, dtype, off):
        return nc.alloc_sbuf_tensor_at(name, shape, dtype, offset=base + off).ap()

    xt_sb = at_("xt", [128, NH * rows], f32r, 0)
    whh_sb = at_("whh", [128, NH * H4], f8e4, 0)
    wih_sb = at_("wih", [128, 2 * HIDDEN], f32r, wih_off)
    xr_sb = at_("xr", [128, 2 * HIDDEN], f32r, xr_off)
    at_sb = at_("at", [128, 2 * rows], f32, blk_off)
    # phase-2 block (aliases the at slots; fenced by the phase barrier)
    o = blk_off
    a_sb = at_("a_t", [128, 4 * 512], f32, o); o += 4 * 512 * 4
    g_sb = at_("g", [128, 2 * 512], f32, o); o += 2 * 512 * 4
    acts_sb = at_("acts", [128, 2 * 512], f32, o); o += 2 * 512 * 4
    tmp_sb = at_("tmp", [128, 512], f32, o); o += 512 * 4
    hst_sb = at_("hst", [128, 2 * 1024], f32, o); o += 2 * 1024 * 4
    ct_sb = at_("ct", [128, 128], f32, o); o += 128 * 4
    ht_sb = at_("ht", [128, 256], bf16, o); o += 512
    assert o - blk_off <= p2_bytes
    def _al(x):
        return (x + 31) // 32 * 32

    o = const_off
    idx_sb = at_("idx_sb", [128, nrt], i32, o); o += _al(nrt * 4)
    bias_sb = at_("bias_sb", [128, NJ], f32, o); o += _al(NJ * 4)
    ident_sb = at_("ident_sb", [128, 128], f32r, o); o += 128 * 4

    with tile.TileContext(nc) as tc:
        # ---------------- Phase 1: gather + transpose + input projection ----
        with (
            tc.tile_pool(name="p1pst", bufs=2, space="PSUM") as pstpool,
            tc.tile_pool(name="p1psm", bufs=4, space="PSUM") as psmpool,
        ):
            nc.sync.dma_start(idx_sb[:], idx_d[:])
            nc.sync.dma_start(bias_sb[:], bias_d[:])
            nc.sync.dma_start(ident_sb[:], ident_d[:])

            for r in range(nrt):
                xr = xr_sb[:, (r % 2) * HIDDEN : (r % 2 + 1) * HIDDEN]
                nc.gpsimd.indirect_dma_start(
                    out=xr,
                    out_offset=None,
                    in_=embed_d[:],
                    in_offset=bass.IndirectOffsetOnAxis(ap=idx_sb[:, r : r + 1], axis=0),
                )
                for kb in range(NH):
                    pt = pstpool.tile([128, 128], f32r, tag="pst")
                    nc.tensor.transpose(
                        pt[:], xr[:, kb * 128 : (kb + 1) * 128], ident_sb[:]
                    )
                    nc.vector.tensor_copy(
                        xt_sb[:, kb * rows + r * 128 : kb * rows + (r + 1) * 128],
                        pt[:],
                    )

            for J in range(NJ):
                wih_t = wih_sb[:, (J % 2) * HIDDEN : (J % 2 + 1) * HIDDEN]
                nc.sync.dma_start(
                    wih_t.rearrange("p (kb j) -> p kb j", j=128),
                    wih_d[:, J * 128 : (J + 1) * 128].rearrange(
                        "(kb p) j -> p kb j", p=128
                    ),
                )
                at_t = at_sb[:, (J % 2) * rows : (J % 2 + 1) * rows]
                for C in range(ncc):
                    pm = psmpool.tile([128, CHUNK], f32, tag="psm")
                    for kb in range(NH):
                        nc.tensor.matmul(
                            pm[:],
                            lhsT=wih_t[:, kb * 128 : (kb + 1) * 128],
                            rhs=xt_sb[
                                :, kb * rows + C * CHUNK : kb * rows + (C + 1) * CHUNK
                            ],
                            start=(kb == 0),
                            stop=(kb == NH - 1),
                        )
                    nc.scalar.activation(
                        at_t[:, C * CHUNK : (C + 1) * CHUNK],
                        pm[:],
                        ACT.Identity,
                        bias=bias_sb[:, J : J + 1],
                        scale=WSCALE,
                    )
                # stage A^T to DRAM, packed gate-major:
                # a_d[t, p, 128*(J//8) + 16*(J%8) + b] = at_t[p, t*CB + b]
                acol = 8 * CB * (J // NH) + CB * (J % NH)
                nc.sync.dma_start(
                    a_d[:, :, acol : acol + CB].rearrange("t p b -> p t b"),
                    at_t.rearrange("p (t b) -> p t b", b=CB),
                )

        # ---------------- Phase 2: LSTM recurrence ---------------------------
        tc.strict_bb_all_engine_barrier()

        with tc.tile_pool(name="p2ps", bufs=1, space="PSUM") as psgpool:
            nc.sync.dma_start(
                whh_sb[:].rearrange("p (kb j) -> p kb j", j=H4),
                whh_d[:].rearrange("(kb p) j -> p kb j", p=128),
            )
            nc.gpsimd.memset(ht_sb[:], 0.0)
            nc.gpsimd.memset(ct_sb[:], 0.0)
            HTW = 128  # bf16 cols per ht buffer

            GW = 8  # steps per output-staging group
            B4 = 4 * CB  # 64 cols per h-tile (4 gates x CB)

            # one PSUM tile per gate type, cols = 16q+b (q-major), reused in
            # place across steps. Gate-major layout lets every elementwise op
            # cover all 8 h-tiles in a single full-width instruction.
            pg = [psgpool.tile([128, 128], f32, name=f"pg{g}", tag=f"pg{g}") for g in range(4)]

            for t in [tt for _rep in range(REPEAT) for tt in range(seq)]:
                hst = hst_sb[:, ((t // GW) % 2) * GW * 128 : ((t // GW) % 2 + 1) * GW * 128]
                a_t = a_sb[:, (t % 4) * 512 : (t % 4 + 1) * 512]
                nc.sync.dma_start(a_t, a_d[t])

                if t > 0:
                    for g in range(4):
                        for q in range(NH):
                            J = 8 * g + q
                            for kb in range(NH):
                                nc.tensor.matmul(
                                    pg[g][:, CB * q : CB * (q + 1)],
                                    lhsT=whh_sb[
                                        :, kb * H4 + J * 128 : kb * H4 + (J + 1) * 128
                                    ],
                                    rhs=ht_sb[:, (t % 2) * HTW + CB * kb : (t % 2) * HTW + CB * (kb + 1)],
                                    start=(kb == 0),
                                    stop=(kb == NH - 1),
                                )
                    gates = g_sb[:, (t % 2) * 512 : (t % 2) * 512 + 512]
                    for g in range(4):
                        nc.vector.tensor_add(
                            gates[:, 128 * g : 128 * (g + 1)],
                            pg[g][:],
                            a_t[:, 128 * g : 128 * (g + 1)],
                        )
                else:
                    gates = a_t
                # gates cols: [0:128]=i, [128:256]=f, [256:384]=g, [384:512]=o
                acts = acts_sb[:, (t % 2) * 512 : (t % 2) * 512 + 512]
                nc.scalar.activation(
                    acts[:, 0:256], gates[:, 0:256], ACT.Sigmoid, scale=1.0 / WSCALE
                )
                nc.scalar.activation(
                    acts[:, 256:384], gates[:, 256:384], ACT.Tanh, scale=1.0 / WSCALE
                )
                nc.scalar.activation(
                    acts[:, 384:512], gates[:, 384:512], ACT.Sigmoid, scale=1.0 / WSCALE
                )

                t1 = tmp_sb[:, 0:128]
                t2 = tmp_sb[:, 128:256]
                nc.vector.tensor_mul(t1, acts[:, 0:128], acts[:, 256:384])
                nc.vector.tensor_mul(t2, acts[:, 128:256], ct_sb[:])
                nc.vector.tensor_add(ct_sb[:], t1, t2)
                tc2 = tmp_sb[:, 256:384]
                nc.scalar.activation(tc2, ct_sb[:], ACT.Tanh)
                hf = hst[:, (t % GW) * 128 : (t % GW) * 128 + 128]
                nc.vector.tensor_mul(hf, acts[:, 384:512], tc2)
                nc.vector.tensor_copy(
                    ht_sb[:, ((t + 1) % 2) * HTW : ((t + 1) % 2) * HTW + 128],
                    hf,
                )

                if t % GW == GW - 1:
                    nc.sync.dma_start(
                        out_d[t - GW + 1 : t + 1].rearrange("t p c -> p t c"),
                        hst.rearrange("p (tt c) -> p tt c", c=128),
                    )

    nc.compile()
    _cache[seq] = nc
    return nc


def _prep_inputs(inputs, seq):
    input_lines = np.asarray(inputs["input_lines"])[:seq]
    embed = np.ascontiguousarray(np.asarray(inputs["embed_input"], dtype=np.float32))
    wihT = np.ascontiguousarray(np.asarray(inputs["W_ih"], np.float32).T)
    whhT = np.ascontiguousarray(
        (np.asarray(inputs["W_hh"], np.float32).T * 64.0).astype(
            ml_dtypes.float8_e4m3
        )
    )
    bias = np.asarray(inputs["b_ih"], np.float32) + np.asarray(inputs["b_hh"], np.float32)
    biasT = np.ascontiguousarray(bias.reshape(NJ, 128).T) * 64.0
    ident = np.eye(128, dtype=np.float32)

    in_maps = []
    for core in range(NCORES):
        sl = input_lines[:, core * CB : (core + 1) * CB].astype(np.int32)
        idx = np.ascontiguousarray(sl.reshape(seq * CB).reshape(-1, 128).T)
        in_maps.append(
            {
                "idx": idx,
                "embed": embed,
                "wihT": wihT,
                "whhT": whhT,
                "biasT": biasT,
                "ident": ident,
            }
        )
    return in_maps


def _assemble(results, seq):
    outs = []
    for core in range(NCORES):
        o = results[core]["out"]  # [seq, 128, 128]: [t, p, 16q+b]
        o = (
            o.reshape(seq, 128, NH, CB)
            .transpose(0, 3, 2, 1)
            .reshape(seq, CB, HIDDEN)
        )
        outs.append(o)
    return np.ascontiguousarray(np.concatenate(outs, axis=1))


def _run(inputs, seq=SEQ):
    from concourse.bass_utils import run_bass_kernel_spmd

    nc = _build(seq)
    in_maps = _prep_inputs(inputs, seq)
    res = run_bass_kernel_spmd(nc, in_maps, core_ids=list(range(NCORES)))
    return _assemble(res.results, seq)


def kernel(input_lines, target_lines, embed_input, W_ih, W_hh, b_ih, b_hh):
    return _run(
        {
            "input_lines": input_lines,
            "embed_input": embed_input,
            "W_ih": W_ih,
            "W_hh": W_hh,
            "b_ih": b_ih,
            "b_hh": b_hh,
        },
        seq=SEQ,
    )

